# revision 18
# baseline (speedup 1.0000x reference)
"""DualAttentionAutoEncoder (DA-RNN) Trainium2 kernel.

Data-parallel over 8 NeuronCores: batch 8192 -> 1024 rows/core.

Key algebraic facts exploited:
  * Encoder input attention is shift-invariant: softmax_d(score_x + (h.wh+c.wc))
    == softmax_d(score_x)  -> attention weights are constant across time and
    independent of the recurrence. So wi_t = at*x_t is precomputed, and the
    encoder loop is a plain LSTM over precomputed inputs u_t.
  * sigmoid(x) = (tanh(x/2)+1)/2 -> only exp+tanh needed (one ACT table set).
  * Decoder y_tilde layer folds into the LSTM gate matmul:
      Wih@(fc_w@[ctx;y] + fc_b) + bih + bhh = W2c@ctx + W2y@y + b'  (host-prepped)
  * Decoder temporal softmax is shift-invariant in b2 -> b2 dropped.

Layouts (per core, B=1024, 8 chunks of 128):
  feature-major state: S1=[h;c] (dec), x_encT [65, 11, B] (row 64 = ones,
  slot t+1 = h_t), gates PSUM [128=(f|i) or (o|g), B].
  batch-major: softmax/ctx path [128b-part, chunk, ...].
"""

import os
import numpy as np
import ml_dtypes

import concourse.bass as bass
import concourse.bacc as bacc
import concourse.mybir as mybir
import concourse.tile as tile

F32 = mybir.dt.float32
F32R = mybir.dt.float32r
BF16 = mybir.dt.bfloat16
FP32 = np.float32
AF = mybir.ActivationFunctionType
ALU = mybir.AluOpType
AX = mybir.AxisListType

H, L, D, OUT = 64, 10, 128, 5
B_FULL = 8192
NCORES = 8


def _prep_weights(inp):
    """Host-side (numpy) preparation of the tiny weight tensors."""
    f = lambda a: np.ascontiguousarray(a, dtype=FP32)
    bf = lambda a: np.ascontiguousarray(a.astype(FP32), dtype=ml_dtypes.bfloat16)
    out = {}

    eye = np.eye(128, dtype=FP32)
    out["eye_f32"] = eye
    out["eye_bf"] = bf(eye)
    out["eye2x64"] = f(np.vstack([np.eye(64, dtype=FP32)] * 2))  # [128,64]
    out["ones_row"] = np.ones((1, 11 * 1024), dtype=FP32)

    # encoder attention: only wx matters (softmax shift-invariance)
    wx = np.asarray(inp["enc_attn_w"], FP32)[0, 2 * H:]            # [10]
    out["wxb"] = f(np.tile(wx[None, :], (128, 1)))                 # [128,10]

    # gate order permutation: torch (i,f,g,o) -> (f,i,o,g)
    perm = np.r_[64:128, 0:64, 192:256, 128:192]

    eW, eU = np.asarray(inp["enc_Wih"], FP32), np.asarray(inp["enc_Whh"], FP32)
    eb = np.asarray(inp["enc_bih"], FP32) + np.asarray(inp["enc_bhh"], FP32)
    out["encWihT"] = bf(eW[perm].T)                                # [128,256] bf16
    out["encWhhTb"] = f(np.vstack([eU[perm].T, eb[perm][None]]))   # [65,256]

    # per-partition ACT scale for psumB=(o|g): tanh(o*0.5), tanh(g*1.0)
    sc = np.ones((128, 1), dtype=FP32)
    sc[:64] = 0.5
    out["sc_og"] = sc

    # decoder attention MLP
    W1 = np.asarray(inp["dec_attn_w1"], FP32)                      # [64,192]
    W1h, W1c, W1x = W1[:, :64], W1[:, 64:128], W1[:, 128:]
    out["decW1"] = f(np.block([[W1h.T, W1h.T], [W1c.T, W1c.T]]))   # [128,128]
    b1 = np.asarray(inp["dec_attn_b1"], FP32)
    out["decb1"] = f(np.concatenate([b1, b1])[:, None])            # [128,1]
    out["decW1xT"] = f(W1x.T)                                      # [64,64]
    w2 = np.asarray(inp["dec_attn_w2"], FP32)[0]                   # [64]
    # w2stk[:, j*10:(j+1)*10]: lhsT for l-pair j -> [10, N] scores psum rows
    # 2j (even lag, z partitions 0:64) and 2j+1 (odd lag, partitions 64:128);
    # all other columns zero so the 5 matmuls accumulate into one psum.
    w2s = np.zeros((128, 50), dtype=FP32)
    for j in range(5):
        w2s[:64, j * 10 + 2 * j] = w2
        w2s[64:, j * 10 + 2 * j + 1] = w2
    out["w2stk"] = bf(w2s)                                         # [128,50] bf16

    # decoder LSTM with folded fc layer
    dW, dU = np.asarray(inp["dec_Wih"], FP32), np.asarray(inp["dec_Whh"], FP32)
    fcw, fcb = np.asarray(inp["fc_w"], FP32), np.asarray(inp["fc_b"], FP32)
    W2c = dW @ fcw[:, :64]                                         # [256,64]
    W2y = dW @ fcw[:, 64:]                                         # [256,5]
    bp = dW @ fcb + np.asarray(inp["dec_bih"], FP32) + np.asarray(inp["dec_bhh"], FP32)
    W2c, W2y, bp, dUp = W2c[perm], W2y[perm], bp[perm], dU[perm]
    # S2 = [ctx(0:64); h(64:128)]
    out["decWg1"] = f(np.vstack([W2c.T, dUp.T]))                   # [128,256]
    out["decWg2"] = f(np.vstack([W2y.T, bp[None]]))                # [6,256]

    fow, fob = np.asarray(inp["fcout_w"], FP32), np.asarray(inp["fcout_b"], FP32)
    out["fcoutT"] = f(np.vstack([fow[:, 64:].T, fow[:, :64].T]))   # [128,5]
    out["fcout_b1"] = f(fob[None, :])                              # [1,5]
    return out


def _r(ap):
    return ap.bitcast(F32R)


def build_module(BC):
    """Build the bass module for per-core batch BC (multiple of 128)."""
    CH = BC // 128
    NH = BC // 512 if BC >= 512 else 1   # number of 512-wide matmul halves
    NW = min(512, BC)                    # matmul moving width
    NQ = max(1, BC // 256)               # 256-wide quarters for z
    QW = min(256, BC)

    nc = bacc.Bacc("TRN2", target_bir_lowering=False, debug=False)

    dt_in = {}

    def din(name, shape, dt=F32):
        dt_in[name] = nc.dram_tensor(name, list(shape), dt, kind="ExternalInput")
        return dt_in[name]

    x_d = din("x", (BC, L, D))
    y_d = din("y_hist", (BC, L, OUT), F32R)
    h0e_d = din("h0_enc", (BC, H))
    c0e_d = din("c0_enc", (BC, H))
    h0d_d = din("h0_dec", (BC, H))
    c0d_d = din("c0_dec", (BC, H))
    eye_d = din("eye_f32", (128, 128))
    eyebf_d = din("eye_bf", (128, 128), BF16)
    eye2_d = din("eye2x64", (128, 64), F32R)
    ones_d = din("ones_row", (1, 11 * 1024), F32R)
    wxb_d = din("wxb", (128, 10))
    encWihT_d = din("encWihT", (128, 256), BF16)
    encWhhTb_d = din("encWhhTb", (65, 256), F32R)
    scog_d = din("sc_og", (128, 1))
    decW1_d = din("decW1", (128, 128), F32R)
    decb1_d = din("decb1", (128, 1))
    decW1xT_d = din("decW1xT", (64, 64), F32R)
    w2stk_d = din("w2stk", (128, 50), BF16)
    decWg1_d = din("decWg1", (128, 256), F32R)
    decWg2_d = din("decWg2", (6, 256), F32R)
    fcoutT_d = din("fcoutT", (128, 5), F32R)
    fcoutb_d = din("fcout_b1", (1, 5), F32R)

    out_d = nc.dram_tensor("out", [BC, OUT], F32, kind="ExternalOutput")

    with tile.TileContext(nc) as tc:
        _emit(nc, tc, dt_in, out_d, BC, CH, NH, NW, NQ, QW)
    nc.compile()
    return nc


def _emit(nc, tc, dd, out_d, BC, CH, NH, NW, NQ, QW):
    from contextlib import ExitStack

    ctx = ExitStack()
    with ctx:
        # ---------- persistent pools ----------
        wpool = ctx.enter_context(tc.tile_pool(name="weights", bufs=1))
        state = ctx.enter_context(tc.tile_pool(name="state", bufs=1))

        def wload(name, shape=None, dt=F32):
            t = wpool.tile(list(shape or dd[name].shape), dt, tag=name)
            nc.sync.dma_start(t[:], dd[name].ap())
            return t

        eye = wload("eye_f32")
        eyebf = wload("eye_bf", dt=BF16)
        eye2 = wload("eye2x64", dt=F32R)
        wxb = wload("wxb")
        encWihT = wload("encWihT", dt=BF16)
        encWhhTb = wload("encWhhTb", dt=F32R)
        scog = wload("sc_og")
        decW1 = wload("decW1", dt=F32R)
        decb1 = wload("decb1")
        decW1xT = wload("decW1xT", dt=F32R)
        w2stk = wload("w2stk", dt=BF16)
        decWg1 = wload("decWg1", dt=F32R)
        decWg2 = wload("decWg2", dt=F32R)
        fcoutT = wload("fcoutT", dt=F32R)
        fcoutb = wload("fcout_b1", dt=F32R)

        # persistent state tensors
        ones_sb = state.tile([1, BC], F32R, tag="ones_sb")
        nc.sync.dma_start(ones_sb[:], dd["ones_row"].ap()[:, :BC])
        x_encT = state.tile([65, L + 1, BC], F32, tag="x_encT")
        uT = state.tile([128, L, BC], BF16, tag="uT")
        xe_bm = state.tile([128, CH, H, L], BF16, tag="xe_bm")   # (c,h,l) l-inner
        pre_bf = state.tile([128, L // 2, BC], BF16, tag="pre_bf")
        z_bf = state.tile([128, L // 2, BC], BF16, tag="z_bf")
        S1 = state.tile([128, BC], F32, tag="S1")    # dec [h; c]
        S2 = state.tile([128, BC], F32, tag="S2")    # dec [ctx; h]

        # ones row of x_encT (row 64) via DMA from ones_row
        nc.sync.dma_start(
            _r(x_encT[64:65, :, :]), dd["ones_row"].ap()[:, : (L + 1) * BC]
        )

        # ---------- setup: load x, h0/c0; compute at, u, uT ----------
        # scratch psum pool for setup transposes
        with tc.tile_pool(name="setup_big", bufs=1) as sb_pool, \
             tc.tile_pool(name="setup_ps", bufs=2, space="PSUM") as sps, \
             tc.tile_pool(name="setup_ps2", bufs=1, space="PSUM") as sps2:

            x_sb = sb_pool.tile([128, CH, L, D], F32, tag="x_sb")
            nc.sync.dma_start(
                x_sb[:], dd["x"].ap().rearrange("(c p) l d -> p c l d", p=128)
            )
            init_bm = {}
            for nm in ("h0_enc", "c0_enc", "h0_dec", "c0_dec"):
                t = sb_pool.tile([128, CH, H], F32, tag=nm)
                nc.sync.dma_start(t[:], dd[nm].ap().rearrange("(c p) h -> p c h", p=128))
                init_bm[nm] = t
            h0e_bm, c0e_bm = init_bm["h0_enc"], init_bm["c0_enc"]
            h0d_bm, c0d_bm = init_bm["h0_dec"], init_bm["c0_dec"]

            # score_x = sum_l wx[l] * x[:,l,:]  (batch-major, per chunk)
            sx = sb_pool.tile([128, CH, D], F32, tag="sx")
            for c in range(CH):
                sxm = sb_pool.tile([128, D, L], F32, tag="sxm")
                nc.vector.tensor_tensor(
                    out=sxm[:].transpose([0, 2, 1]),
                    in0=x_sb[:, c, :, :],
                    in1=wxb[:].unsqueeze(2).broadcast_to([128, L, D]),
                    op=ALU.mult,
                )
                nc.vector.tensor_reduce(
                    out=sx[:, c, :], in_=sxm[:], axis=AX.X, op=ALU.add
                )

            # at = softmax_d(score_x): exp per chunk with fused sum
            e_at = sb_pool.tile([128, CH, D], F32, tag="e_at")
            Ssum = sb_pool.tile([128, CH], F32, tag="Ssum")
            for c in range(CH):
                nc.scalar.activation(
                    e_at[:, c, :], sx[:, c, :], AF.Exp,
                    accum_out=Ssum[:, c : c + 1],
                )
            rS = sb_pool.tile([128, CH], F32, tag="rS")
            nc.vector.reciprocal(rS[:], Ssum[:])
            at = sb_pool.tile([128, CH, D], F32, tag="at")
            nc.vector.tensor_tensor(
                out=at[:], in0=e_at[:],
                in1=rS[:].unsqueeze(2).broadcast_to([128, CH, D]),
                op=ALU.mult,
            )
            # u_c = at_c * x_c (broadcast over lag), then uT via PE transposes
            for c in range(CH):
                u_c = sb_pool.tile([128, L, D], F32, tag="u_c")
                nc.vector.tensor_tensor(
                    out=u_c[:], in0=x_sb[:, c, :, :],
                    in1=at[:, c, :].unsqueeze(1).broadcast_to([128, L, D]),
                    op=ALU.mult,
                )
                psU = sps.tile([128, L, 128], F32, tag="psU")
                for t in range(L):
                    nc.tensor.transpose(psU[:, t, :], u_c[:, t, :], eye[:])
                if c % 2 == 0:
                    nc.vector.tensor_copy(
                        uT[:, :, c * 128 : (c + 1) * 128], psU[:]
                    )
                else:
                    nc.scalar.copy(
                        uT[:, :, c * 128 : (c + 1) * 128], psU[:]
                    )

            # h0/c0 transposes -> x_encT slot 0, S1, S2 (+ initial c psums later)
            psH = sps2.tile([64, CH, 128], F32, tag="psH")
            for c in range(CH):
                nc.tensor.transpose(psH[:, c, :], h0e_bm[:, c, :], eye[:])
            nc.vector.tensor_copy(
                _r(x_encT[0:64, 0, :]), psH[:].rearrange("p c b -> p (c b)")
            )
            psH2 = sps2.tile([64, CH, 128], F32, tag="psH")
            for c in range(CH):
                nc.tensor.transpose(psH2[:, c, :], h0d_bm[:, c, :], eye[:])
            nc.vector.tensor_copy(_r(S1[0:64, :]), psH2[:].rearrange("p c b -> p (c b)"))
            nc.scalar.copy(_r(S2[64:128, :]), psH2[:].rearrange("p c b -> p (c b)"))

            # keep c0 batch-major tiles alive into the loops via copies to state
            c0e_T = state.tile([64, BC], F32, tag="c0e_T")
            psC0 = sps2.tile([64, CH, 128], F32, tag="psH")
            for c in range(CH):
                nc.tensor.transpose(psC0[:, c, :], c0e_bm[:, c, :], eye[:])
            nc.vector.tensor_copy(c0e_T[:], psC0[:].rearrange("p c b -> p (c b)"))
            c0d_T = state.tile([64, BC], F32, tag="c0d_T")
            psC1 = sps2.tile([64, CH, 128], F32, tag="psH")
            for c in range(CH):
                nc.tensor.transpose(psC1[:, c, :], c0d_bm[:, c, :], eye[:])
            nc.vector.tensor_copy(c0d_T[:], psC1[:].rearrange("p c b -> p (c b)"))
            nc.scalar.copy(_r(S1[64:128, :]), psC1[:].rearrange("p c b -> p (c b)"))

        # ---------- encoder loop ----------
        ework = ctx.enter_context(tc.tile_pool(name="ework", bufs=1))
        Tfi = ework.tile([128, BC], F32, tag="Tfi")
        Tog = ework.tile([128, BC], F32, tag="Tog")
        S1g = ework.tile([128, BC], F32, tag="S1g")
        Msb = ework.tile([128, BC], F32, tag="Msb")
        S2a = ework.tile([64, BC], F32, tag="S2a")
        thc = ework.tile([64, BC], F32, tag="thc")

        with tc.tile_pool(name="enc_g", bufs=1, space="PSUM") as pg, \
             tc.tile_pool(name="enc_c", bufs=1, space="PSUM") as pc, \
             tc.tile_pool(name="enc_xe", bufs=2, space="PSUM") as pxe:

            psC_prev = None
            for t in range(L):
                psA = pg.tile([128, NH, NW], F32, tag="gA")
                psB = pg.tile([128, NH, NW], F32, tag="gB")
                for m, ps in ((0, psA), (1, psB)):
                    lhs_u = encWihT[:, m * 128 : (m + 1) * 128]
                    lhs_h = encWhhTb[:, m * 128 : (m + 1) * 128]
                    for hf in range(NH):
                        sl = slice(hf * NW, (hf + 1) * NW)
                        nc.tensor.matmul(
                            ps[:, hf, :], lhs_u, uT[:, t, sl],
                            start=True, stop=False,
                        )
                        nc.tensor.matmul(
                            ps[:, hf, :], lhs_h, _r(x_encT[0:65, t, sl]),
                            start=False, stop=True,
                        )
                pAv = psA[:].rearrange("p h w -> p (h w)")
                pBv = psB[:].rearrange("p h w -> p (h w)")
                nc.scalar.activation(Tfi[:], pAv, AF.Tanh, scale=0.5)
                nc.scalar.activation(Tog[:], pBv, AF.Tanh, scale=scog[:, 0:1])
                nc.vector.tensor_scalar(
                    out=S1g[:], in0=Tfi[:], scalar1=0.5, scalar2=0.5,
                    op0=ALU.mult, op1=ALU.add,
                )
                # m2 = sig(i)*tanh(g) -> Msb[0:64];  m1 = sig(f)*c -> Msb[64:128]
                nc.vector.tensor_tensor(
                    out=_r(Msb[0:64, :]), in0=S1g[64:128, :], in1=Tog[64:128, :],
                    op=ALU.mult,
                )
                nc.vector.tensor_tensor(
                    out=_r(Msb[64:128, :]), in0=S1g[0:64, :],
                    in1=(c0e_T[:] if t == 0 else psC_prev[:]),
                    op=ALU.mult,
                )
                psC = pc.tile([64, BC], F32, tag="c")
                for hf in range(NH):
                    sl = slice(hf * NW, (hf + 1) * NW)
                    nc.tensor.matmul(
                        psC[:, sl], eye2[:], _r(Msb[:, sl]),
                        start=True, stop=True,
                    )
                nc.scalar.activation(thc[:], psC[:], AF.Tanh)
                nc.vector.tensor_scalar(
                    out=S2a[:], in0=Tog[0:64, :], scalar1=0.5, scalar2=0.5,
                    op0=ALU.mult, op1=ALU.add,
                )
                nc.vector.tensor_tensor(
                    out=_r(x_encT[0:64, t + 1, :]), in0=S2a[:], in1=thc[:],
                    op=ALU.mult,
                )
                # batch-major copy of h_t for decoder ctx (bf16, l-innermost)
                psXE = pxe.tile([128, CH, H], F32, tag="xe")
                for c in range(CH):
                    nc.tensor.transpose(
                        psXE[:, c, :],
                        x_encT[0:64, t + 1, c * 128 : (c + 1) * 128],
                        eye[0:64, 0:64],
                    )
                nc.scalar.copy(
                    xe_bm[:, :, :, t], psXE[:]
                )
                psC_prev = psC

        # ---------- decoder pre = x_enc @ W1x.T (feature-major) ----------
        dwork = ctx.enter_context(tc.tile_pool(name="dwork", bufs=1))
        with tc.tile_pool(name="dec_pre", bufs=1, space="PSUM") as pp:
            for q in range(NQ):
                sl = slice(q * QW, (q + 1) * QW)
                psP = pp.tile([64, 2, L // 2, QW], F32, tag="pre")
                for j in range(L // 2):
                    for par in range(2):
                        nc.tensor.matmul(
                            psP[:, par, j, :],
                            decW1xT[:],
                            _r(x_encT[0:64, 1 + 2 * j + par, sl]),
                            start=True, stop=True,
                        )
                nc.vector.tensor_copy(pre_bf[0:64, :, sl], psP[:, 0, :, :])
                nc.scalar.copy(pre_bf[64:128, :, sl], psP[:, 1, :, :])

        # ---------- decoder loop ----------
        sc_sb = dwork.tile([10, BC], F32, tag="sc_sb")
        e_bf = dwork.tile([128, CH, L], BF16, tag="e_bf")
        Ssm = dwork.tile([128, CH], F32, tag="Ssm")
        rSd = dwork.tile([128, CH], F32, tag="rSd")
        at_bf = dwork.tile([128, CH, L], BF16, tag="at_bf")
        cm_bf = dwork.tile([128, CH, H, L], BF16, tag="cm_bf")
        ctx_f = dwork.tile([128, CH, H], F32, tag="ctx_f")
        out_sb = dwork.tile([5, BC], F32, tag="out_sb")

        with tc.tile_pool(name="dec_ps", bufs=2, space="PSUM") as dps, \
             tc.tile_pool(name="dec_c", bufs=1, space="PSUM") as dpc, \
             tc.tile_pool(name="ypool", bufs=2) as ypool:

            psCd_prev = None
            for t in range(L):
                yt = ypool.tile([6, BC], F32R, tag="yt")
                nc.sync.dma_start(
                    yt[0:5, :], dd["y_hist"].ap()[:, t, :].rearrange("b o -> o b")
                )
                nc.sync.dma_start(yt[5:6, :], dd["ones_row"].ap()[:, :BC])
                # z = tanh(pre + q2 + b1), feature-major [128=(l%2,h), 5, B]
                for q in range(NQ):
                    sl = slice(q * QW, (q + 1) * QW)
                    zq = dps.tile([128, L // 2, QW], F32, tag="big")
                    for j in range(L // 2):
                        nc.tensor.matmul(
                            zq[:, j, :], decW1[:], _r(S1[:, sl]),
                            start=True, stop=False,
                        )
                        nc.tensor.matmul(
                            zq[:, j, :], eyebf[:], pre_bf[:, j, sl],
                            start=False, stop=True,
                        )
                    nc.scalar.activation(
                        z_bf[:, :, sl], zq[:], AF.Tanh, bias=decb1[:, 0:1]
                    )
                # scores = z @ w2 -> psum [10, B]
                psS = dps.tile([10, NH, NW], F32, tag="big")
                for hf in range(NH):
                    sl = slice(hf * NW, (hf + 1) * NW)
                    for j in range(L // 2):
                        nc.tensor.matmul(
                            psS[:, hf, :], w2stk[:, j * 10 : (j + 1) * 10],
                            z_bf[:, j, sl],
                            start=(j == 0), stop=(j == L // 2 - 1),
                        )
                nc.vector.tensor_copy(sc_sb[:], psS[:].rearrange("p h w -> p (h w)"))
                psT = dps.tile([128, CH, L], F32, tag="big")
                for c in range(CH):
                    nc.tensor.transpose(
                        psT[:, c, :], sc_sb[:, c * 128 : (c + 1) * 128],
                        eye[0:10, 0:10],
                    )
                nc.scalar.activation(e_bf[:], psT[:], AF.Exp)
                nc.vector.tensor_reduce(out=Ssm[:], in_=e_bf[:], axis=AX.X, op=ALU.add)
                nc.vector.reciprocal(rSd[:], Ssm[:])
                nc.vector.tensor_tensor(
                    out=at_bf[:], in0=e_bf[:],
                    in1=rSd[:].unsqueeze(2).broadcast_to([128, CH, L]),
                    op=ALU.mult,
                )
                # ctx = sum_l at_l * x_enc_l  (batch-major, l innermost)
                nc.vector.tensor_tensor(
                    out=cm_bf[:], in0=xe_bm[:],
                    in1=at_bf[:].unsqueeze(2).broadcast_to([128, CH, H, L]),
                    op=ALU.mult,
                )
                nc.vector.tensor_reduce(out=ctx_f[:], in_=cm_bf[:], axis=AX.X, op=ALU.add)
                psCT = dps.tile([64, CH, 128], F32, tag="big")
                for c in range(CH):
                    nc.tensor.transpose(psCT[:, c, :], ctx_f[:, c, :], eye[:])
                nc.scalar.copy(_r(S2[0:64, :]), psCT[:].rearrange("p c b -> p (c b)"))

                # gates
                psA = dps.tile([128, NH, NW], F32, tag="big")
                psB = dps.tile([128, NH, NW], F32, tag="big")
                for m, ps in ((0, psA), (1, psB)):
                    lhs1 = decWg1[:, m * 128 : (m + 1) * 128]
                    lhs2 = decWg2[:, m * 128 : (m + 1) * 128]
                    for hf in range(NH):
                        sl = slice(hf * NW, (hf + 1) * NW)
                        nc.tensor.matmul(
                            ps[:, hf, :], lhs1, _r(S2[:, sl]),
                            start=True, stop=False,
                        )
                        nc.tensor.matmul(
                            ps[:, hf, :], lhs2, yt[0:6, sl],
                            start=False, stop=True,
                        )
                pAv = psA[:].rearrange("p h w -> p (h w)")
                pBv = psB[:].rearrange("p h w -> p (h w)")
                nc.scalar.activation(Tfi[:], pAv, AF.Tanh, scale=0.5)
                nc.scalar.activation(Tog[:], pBv, AF.Tanh, scale=scog[:, 0:1])
                nc.vector.tensor_scalar(
                    out=S1g[:], in0=Tfi[:], scalar1=0.5, scalar2=0.5,
                    op0=ALU.mult, op1=ALU.add,
                )
                nc.vector.tensor_tensor(
                    out=_r(Msb[0:64, :]), in0=S1g[64:128, :], in1=Tog[64:128, :],
                    op=ALU.mult,
                )
                nc.vector.tensor_tensor(
                    out=_r(Msb[64:128, :]), in0=S1g[0:64, :],
                    in1=(c0d_T[:] if t == 0 else psCd_prev[:]),
                    op=ALU.mult,
                )
                psCd = dpc.tile([64, BC], F32, tag="cd")
                for hf in range(NH):
                    sl = slice(hf * NW, (hf + 1) * NW)
                    nc.tensor.matmul(
                        psCd[:, sl], eye2[:], _r(Msb[:, sl]),
                        start=True, stop=True,
                    )
                # c also needed in S1[64:128] for next q2 matmul
                nc.vector.tensor_copy(_r(S1[64:128, :]), psCd[:])
                nc.scalar.activation(thc[:], psCd[:], AF.Tanh)
                nc.vector.tensor_scalar(
                    out=S2a[:], in0=Tog[0:64, :], scalar1=0.5, scalar2=0.5,
                    op0=ALU.mult, op1=ALU.add,
                )
                nc.vector.tensor_tensor(
                    out=_r(S1[0:64, :]), in0=S2a[:], in1=thc[:], op=ALU.mult
                )
                nc.vector.tensor_copy(_r(S2[64:128, :]), S1[0:64, :])
                psCd_prev = psCd

            # out = [h, ctx] @ fcout_w.T + fcout_b
            psO = dps.tile([5, NH, NW], F32, tag="big")
            for hf in range(NH):
                sl = slice(hf * NW, (hf + 1) * NW)
                nc.tensor.matmul(
                    psO[:, hf, :], fcoutT[:], _r(S2[:, sl]),
                    start=True, stop=False,
                )
                nc.tensor.matmul(
                    psO[:, hf, :], fcoutb[:], ones_sb[:, sl],
                    start=False, stop=True,
                )
            nc.vector.tensor_copy(out_sb[:], psO[:].rearrange("p h w -> p (h w)"))
            nc.sync.dma_start(out_d.ap().rearrange("b o -> o b"), out_sb[:])


_BUILD_CACHE = {}


def _get_module(BC):
    if BC not in _BUILD_CACHE:
        _BUILD_CACHE[BC] = build_module(BC)
    return _BUILD_CACHE[BC]


def kernel(**inputs):
    from concourse.bass_utils import run_bass_kernel_spmd

    B = inputs["x"].shape[0]
    BC = B // NCORES
    nc = _get_module(BC)
    prep = _prep_weights(inputs)

    data_keys = ["x", "y_hist", "h0_enc", "c0_enc", "h0_dec", "c0_dec"]
    in_maps = []
    for c in range(NCORES):
        sl = slice(c * BC, (c + 1) * BC)
        m = {k: np.ascontiguousarray(np.asarray(inputs[k], FP32)[sl]) for k in data_keys}
        m.update(prep)
        in_maps.append(m)

    res = run_bass_kernel_spmd(nc, in_maps, list(range(NCORES)))
    out = np.concatenate([r["out"] for r in res.results], axis=0)
    return np.ascontiguousarray(out, dtype=FP32)


if __name__ == "__main__":
    nc = build_module(1024)
    print("built OK:", len(nc.m.functions[0].instructions) if hasattr(nc.m.functions[0], "instructions") else "?")
